# revision 1
# baseline (speedup 1.0000x reference)
"""Multi-head causal attention (B=2, S=2048, D=1024, H=16) on 8 TRN2 NeuronCores.

Sharding: tensor-parallel over heads. Core c owns heads [2c, 2c+1]:
  - Wq/Wk/Wv column-shard [1024, 128] (2 heads x 64)
  - Wo row-shard [128, 1024]
Each core computes a partial output [2, 2048, 1024]; host sums partials + bo.

Per-core algorithm (all matmul operands in float32r for full-rate PE):
  phase 0: X^T via PE transpose      XT[sj] [din=128p, 8ko, 512s]
  phase 1: QT/KT/VT = W^T X^T        [128(2h x 64), 512] per chunk;
           V natural via PE transpose of VT, ones column appended per head
  phase 2: per head: scoresT = KT^T QT (k on partitions), exp on ACT
           (scale=1/8 folded in), causal mask applied multiplicatively
           post-exp on GpSimd (only diagonal blocks, only live columns),
           PV with ones-column -> ctxT[64] + denominator row,
           reciprocal + PE broadcast -> normalize ctxT.
  phase 3: out[s, :] = ctxT^T @ Wo_shard   (both heads stacked, K=128)
"""

import numpy as np

B, S, D = 2, 2048, 1024
H_PER_CORE = 2
HD = 64
DM = H_PER_CORE * HD  # 128, per-core model-dim shard
N_CORES = 8
P = 128
QT_TILE = 512          # q free-dim tile in attention

_BUILD_CACHE = {}


def build_bass(mm_mode: str = "fp32r"):
    """Build the per-core Bass program. mm_mode in {fp32r, fp32}."""
    import contextlib

    import concourse.tile as tile
    from concourse import bacc, mybir
    from concourse.masks import make_identity

    f32 = mybir.dt.float32
    f16 = mybir.dt.float16
    f32r = mybir.dt.float32r if mm_mode == "fp32r" else mybir.dt.float32
    Exp = mybir.ActivationFunctionType.Exp
    mult_op = mybir.AluOpType.mult

    nc = bacc.Bacc("TRN2", target_bir_lowering=False, debug=False)

    X = nc.dram_tensor("X", [B, S, D], f32, kind="ExternalInput").ap()
    Wq = nc.dram_tensor("Wq", [D, DM], f32, kind="ExternalInput").ap()
    Wk = nc.dram_tensor("Wk", [D, DM], f32, kind="ExternalInput").ap()
    Wv = nc.dram_tensor("Wv", [D, DM], f32, kind="ExternalInput").ap()
    Wo = nc.dram_tensor("Wo", [DM, D], f32, kind="ExternalInput").ap()
    Out = nc.dram_tensor("Out", [B, S, D], f16, kind="ExternalOutput").ap()

    KO = D // P            # 8 contraction chunks for projections
    NSI = S // P           # 16 s-chunks of 128
    NSJ = S // QT_TILE     # 4 s-chunks of 512

    lp_ctx = (nc.allow_low_precision(reason="float32r rounding is intentional")
              if hasattr(nc, "allow_low_precision") else contextlib.nullcontext())
    with lp_ctx, tile.TileContext(nc) as tc:
        with tc.tile_pool(name="consts", bufs=1) as consts, \
             tc.tile_pool(name="wpool", bufs=1) as wpool, \
             tc.tile_pool(name="xt", bufs=1) as xtp, \
             tc.tile_pool(name="xn", bufs=8) as xnp, \
             tc.tile_pool(name="qkv", bufs=1) as qkvp, \
             tc.tile_pool(name="expt", bufs=16) as exptp, \
             tc.tile_pool(name="rbc", bufs=3) as rbcp, \
             tc.tile_pool(name="ctx", bufs=1) as ctxp, \
             tc.tile_pool(name="den", bufs=3) as denp, \
             tc.tile_pool(name="outp", bufs=6) as outp, \
             tc.tile_pool(name="psum", bufs=2, space="PSUM") as psum:

            # ---- constants ----
            ident_f32 = consts.tile([P, P], f32, tag="ident_f32")
            make_identity(nc, ident_f32[:])
            ident = consts.tile([P, P], f32r, tag="ident")
            nc.vector.tensor_copy(out=ident[:], in_=ident_f32[:])

            ones_col = consts.tile([P, 1], f32, tag="ones_col")
            nc.vector.memset(ones_col[:], 1.0)

            # ---- weights (cast to f32r via gpsimd DMA), loaded after the
            # first batch of X tiles so phase 0 starts immediately ----
            def load_xn(b, si):
                xn = xnp.tile([P, KO, P], f32r, tag="xn", name="xn")
                nc.gpsimd.dma_start(
                    xn[:],
                    X[b, si * P:(si + 1) * P, :]
                    .rearrange("s (ko p) -> s ko p", p=P),
                )
                return xn

            first_xns = [load_xn(0, t) for t in range(4)]

            def load_w(ap, name):
                t = wpool.tile([P, KO, DM], f32r, tag=name)
                nc.gpsimd.dma_start(t[:], ap.rearrange("(ko p) m -> p ko m", p=P))
                return t

            Wq_sb = load_w(Wq, "wq")
            Wk_sb = load_w(Wk, "wk")
            Wv_sb = load_w(Wv, "wv")
            Wo_sb = wpool.tile([DM, D], f32r, tag="wo")
            nc.gpsimd.dma_start(Wo_sb[:], Wo[:])

            # per-batch state
            XT = {0: {}, 1: {}}
            QKV = {0: {}, 1: {}}   # (nm, sj) -> tile
            V = {0: {}, 1: {}}
            CTX = {0: {}, 1: {}}

            def ph0_sj(b, sj):
                """Transpose X chunk sj of batch b into XT[b][sj]."""
                xt = xtp.tile([P, KO, QT_TILE], f32r, tag=f"xt{sj}",
                              name=f"xt{sj}")
                XT[b][sj] = xt
                if b == 0 and sj == 0:
                    xns = first_xns
                else:
                    xns = [load_xn(b, sj * 4 + t) for t in range(4)]
                for ko in range(KO):
                    ps = psum.tile([P, QT_TILE], f32r, tag="tr", name="ps_tr")
                    for t in range(4):
                        nc.tensor.transpose(
                            ps[:, t * P:(t + 1) * P], xns[t][:, ko, :], ident[:]
                        )
                    nc.vector.tensor_copy(out=xt[:, ko, :], in_=ps[:])

            def ph1_proj(b, sj):
                """QT/KT/VT projections for chunk sj."""
                for w, nm in ((Wq_sb, "qt"), (Wk_sb, "kt"), (Wv_sb, "vt")):
                    dst = qkvp.tile([DM, QT_TILE], f32r, tag=f"{nm}{sj}",
                                    name=f"{nm}{sj}")
                    QKV[b][(nm, sj)] = dst
                    ps = psum.tile([P, QT_TILE], f32, tag="prj", name="ps_prj")
                    for ko in range(KO):
                        nc.tensor.matmul(
                            ps[:], w[:, ko, :], XT[b][sj][:, ko, :],
                            start=(ko == 0), stop=(ko == KO - 1),
                        )
                    nc.vector.tensor_copy(out=dst[:], in_=ps[:])

            def ph1_v(b, si):
                """V natural chunk si via PE transpose of VT.
                V[si][:, 0:64]=h0, col 64=1; cols 65:129=h1, col 129=1."""
                v = qkvp.tile([P, 2 * (HD + 1)], f32r, tag=f"v{si}",
                              name=f"v{si}")
                V[b][si] = v
                if b == 0:
                    nc.vector.tensor_copy(out=v[:, HD:HD + 1], in_=ones_col[:])
                    nc.vector.tensor_copy(
                        out=v[:, 2 * HD + 1:2 * HD + 2], in_=ones_col[:]
                    )
                ps = psum.tile([P, QT_TILE], f32r, tag="prj", name="ps_v")
                nc.tensor.transpose(
                    ps[:, :P],
                    QKV[b][("vt", si // 4)][:, (si % 4) * P:(si % 4 + 1) * P],
                    ident[:],
                )
                nc.vector.tensor_copy(out=v[:, 0:HD], in_=ps[:, 0:HD])
                nc.vector.tensor_copy(
                    out=v[:, HD + 1:2 * HD + 1], in_=ps[:, HD:DM]
                )

            def attn_qj(b, qj):
                """Attention for q-chunk qj (both heads), 1-deep skewed:
                scores(ki+1) issue before PV(ki) so exp latency hides."""
                ctx = ctxp.tile([DM, QT_TILE], f32r, tag=f"ctx{qj}",
                                name=f"ctx{qj}")
                CTX[b][qj] = ctx
                nk = 4 * qj + 4
                ctx_ps = {}
                for h in range(H_PER_CORE):
                    ctx_ps[h] = psum.tile(
                        [P, QT_TILE], f32, tag="ctx", name=f"ctx_ps{h}"
                    )
                ets = {}

                def emit_scores(ki):
                    # diagonal block j: columns < 128j are fully masked,
                    # restrict all work to live columns [128j, 512)
                    j = ki - 4 * qj
                    col0 = max(0, j) * P
                    w = QT_TILE - col0
                    for h in range(H_PER_CORE):
                        hp = slice(h * HD, (h + 1) * HD)
                        s_ps = psum.tile([P, QT_TILE], f32, tag="s",
                                         name="s_ps")
                        nc.tensor.matmul(
                            s_ps[:, col0:],
                            QKV[b][("kt", ki // 4)][
                                hp, (ki % 4) * P:(ki % 4 + 1) * P],
                            QKV[b][("qt", qj)][hp, col0:],
                            start=True, stop=True,
                        )
                        et = exptp.tile([P, QT_TILE], f32r, tag="et", name="et")
                        nc.scalar.activation(
                            et[:, col0:], s_ps[:, col0:], Exp, scale=0.125
                        )
                        if j >= 0:
                            # zero upper-triangular (k > q): masked iff kp > qf.
                            # only columns [col0, col0+128) can be masked
                            nc.gpsimd.affine_select(
                                out=et[:, col0:col0 + P],
                                in_=et[:, col0:col0 + P],
                                compare_op=mybir.AluOpType.is_ge,
                                fill=0.0, base=0,
                                pattern=[[1, P]],
                                channel_multiplier=-1,
                            )
                        ets[(ki, h)] = (et, col0)

                def emit_pv(ki):
                    for h in range(H_PER_CORE):
                        et, col0 = ets.pop((ki, h))
                        nc.tensor.matmul(
                            ctx_ps[h][:HD + 1, col0:],
                            V[b][ki][:, h * (HD + 1):(h + 1) * (HD + 1)],
                            et[:, col0:],
                            start=(ki == 0), stop=(ki == nk - 1),
                        )

                for ki in range(nk):
                    emit_scores(ki)
                    emit_pv(ki)

                for h in range(H_PER_CORE):
                    hp = slice(h * HD, (h + 1) * HD)
                    den = denp.tile([1, QT_TILE], f32r, tag="den", name="den")
                    nc.vector.reciprocal(den[:], ctx_ps[h][HD:HD + 1, :])
                    rbc = rbcp.tile([HD, QT_TILE], f32r, tag="rbc", name="rbc")
                    nc.gpsimd.partition_broadcast(rbc[:], den[:])
                    nc.vector.tensor_tensor(
                        ctx[hp, :], ctx_ps[h][:HD, :], rbc[:], mult_op
                    )

            def ph3_qj(b, qj):
                """Output projection for the 4 s-tiles of q-chunk qj."""
                for st in range(4 * qj, 4 * qj + 4):
                    for dj in range(2):
                        ps = psum.tile([P, QT_TILE], f32, tag="prj",
                                       name="ps_out")
                        nc.tensor.matmul(
                            ps[:],
                            CTX[b][qj][:, (st % 4) * P:(st % 4 + 1) * P],
                            Wo_sb[:, dj * QT_TILE:(dj + 1) * QT_TILE],
                            start=True, stop=True,
                        )
                        ot = outp.tile([P, QT_TILE], f16, tag="ot", name="ot")
                        nc.vector.tensor_copy(out=ot[:], in_=ps[:])
                        nc.scalar.dma_start(
                            Out[b, st * P:(st + 1) * P,
                                dj * QT_TILE:(dj + 1) * QT_TILE],
                            ot[:],
                        )

            # ---- software-pipelined emission across the two batches ----
            # sequential per-batch emission; the Tile scheduler overlaps
            # batches through the split per-purpose psum/sbuf slot groups
            for b in range(B):
                for sj in range(NSJ):
                    ph0_sj(b, sj)
                for sj in range(NSJ):
                    ph1_proj(b, sj)
                for si in range(NSI):
                    ph1_v(b, si)
                for qj in range(NSJ):
                    attn_qj(b, qj)
                for qj in range(NSJ):
                    ph3_qj(b, qj)

    nc.compile()
    return nc


def _get_nc(mm_mode: str = "fp32r"):
    if mm_mode not in _BUILD_CACHE:
        _BUILD_CACHE[mm_mode] = build_bass(mm_mode)
    return _BUILD_CACHE[mm_mode]


def kernel(X, Wq, Wk, Wv, Wo, bo, mm_mode: str = "fp32r"):
    from concourse.bass_utils import run_bass_kernel_spmd

    X = np.ascontiguousarray(np.asarray(X, dtype=np.float32))
    Wq = np.asarray(Wq, dtype=np.float32)
    Wk = np.asarray(Wk, dtype=np.float32)
    Wv = np.asarray(Wv, dtype=np.float32)
    Wo = np.asarray(Wo, dtype=np.float32)
    bo = np.asarray(bo, dtype=np.float32)

    nc = _get_nc(mm_mode)

    in_maps = []
    for c in range(N_CORES):
        cs = slice(c * DM, (c + 1) * DM)
        in_maps.append({
            "X": X,
            "Wq": np.ascontiguousarray(Wq[:, cs]),
            "Wk": np.ascontiguousarray(Wk[:, cs]),
            "Wv": np.ascontiguousarray(Wv[:, cs]),
            "Wo": np.ascontiguousarray(Wo[cs, :]),
        })

    res = run_bass_kernel_spmd(nc, in_maps, core_ids=list(range(N_CORES)))
    out = np.zeros((B, S, D), dtype=np.float64)
    for c in range(N_CORES):
        out += res.results[c]["Out"].astype(np.float64)
    out += bo.astype(np.float64)
    return out.astype(np.float32)



# revision 2
# speedup vs baseline: 1.2814x; 1.2814x over previous
"""Multi-head causal attention (B=2, S=2048, D=1024, H=16) on 8 TRN2 NeuronCores.

Sharding: batch x head-group tensor parallel. Core c owns batch c//4 and
heads [4*(c%4), 4*(c%4)+4) (a DM=256 model-dim shard). The host
pre-transposes X per batch to XT [D, S] and casts all device inputs to
bf16; each core computes a partial output [S, D] for its batch; the host
sums the 4 partials per batch and adds bo.

Per-core program (all matmul operands bf16, PSUM accumulate f32):
  QT/KT = W^T XT    [256, 2048], m on partitions (2 groups of 128)
  V     = XT^T Wv   [2048, 256] natural (s on partitions), stored per
          head with an appended ones column (softmax denominator trick)
  attention per q-chunk (512) per head pair: scoresT = K Q^T on PE,
  exp on ACT (scale=1/8, both heads of the pair in one activation off a
  2-bank psum tile), causal diagonal-block mask applied multiplicatively
  post-exp on DVE (triangular constant, bf16 4x mode), PV with the ones
  column -> ctxT[64] + denominator row, reciprocal + gpsimd
  partition_broadcast -> normalized ctxT in bf16.
  out[s, :] = ctxT^T Wo (2 dm-chunks accumulated), f16 copy, DMA out.

Emission interleaves projection / out-projection psum groups into the
attention ki-loops (skew-1 scores->PV) so the in-order PE queue never
stalls on the exp->PV dependency.
"""

import numpy as np

B, S, D = 2, 2048, 1024
H_PER_CORE = 4
HD = 64
DM = H_PER_CORE * HD   # 256, per-core model-dim shard
N_CORES = 8
P = 128
QT_TILE = 512          # q free-dim tile in attention
KO = D // P            # 8 contraction chunks for projections
NSI = S // P           # 16 s-chunks of 128
NSJ = S // QT_TILE     # 4 s-chunks of 512

_BUILD_CACHE = {}


def build_bass(mm_mode: str = "bf16"):
    """Build the per-core Bass program. mm_mode in {bf16, fp32r}."""
    import contextlib

    import concourse.tile as tile
    from concourse import bacc, mybir
    from concourse.masks import make_upper_triangular

    f32 = mybir.dt.float32
    f16 = mybir.dt.float16
    DT = mybir.dt.bfloat16 if mm_mode == "bf16" else mybir.dt.float32r
    Exp = mybir.ActivationFunctionType.Exp
    mult_op = mybir.AluOpType.mult

    nc = bacc.Bacc("TRN2", target_bir_lowering=False, debug=False)

    XTd = nc.dram_tensor("XT", [D, S], DT, kind="ExternalInput").ap()
    Wq = nc.dram_tensor("Wq", [D, DM], DT, kind="ExternalInput").ap()
    Wk = nc.dram_tensor("Wk", [D, DM], DT, kind="ExternalInput").ap()
    Wv = nc.dram_tensor("Wv", [D, DM], DT, kind="ExternalInput").ap()
    Wo = nc.dram_tensor("Wo", [DM, D], DT, kind="ExternalInput").ap()
    Out = nc.dram_tensor("Out", [S, D], f16, kind="ExternalOutput").ap()

    lp_ctx = (nc.allow_low_precision(reason="bf16 rounding is intentional")
              if hasattr(nc, "allow_low_precision") else contextlib.nullcontext())
    with lp_ctx, tile.TileContext(nc) as tc:
        with tc.tile_pool(name="consts", bufs=1) as consts, \
             tc.tile_pool(name="wpool", bufs=1) as wpool, \
             tc.tile_pool(name="qt", bufs=2) as qtp, \
             tc.tile_pool(name="et", bufs=4) as etp, \
             tc.tile_pool(name="ctx", bufs=2) as ctxp, \
             tc.tile_pool(name="den", bufs=4) as denp, \
             tc.tile_pool(name="rbc", bufs=3) as rbcp, \
             tc.tile_pool(name="outp", bufs=4) as outp, \
             tc.tile_pool(name="ps_mm", bufs=2, space="PSUM") as ps_mm, \
             tc.tile_pool(name="ps_s", bufs=2, space="PSUM") as ps_s, \
             tc.tile_pool(name="ps_ctx", bufs=2, space="PSUM") as ps_ctx:

            # ---- constants ----
            # tri[k, q] = 1 where k <= q else 0 (keep-mask for the causal
            # diagonal 128x128 block of scoresT)
            tri = consts.tile([P, P], DT, tag="tri")
            make_upper_triangular(nc, tri[:], val=1.0, diag=True)

            # ---- persistent sbuf tensors ----
            xt = wpool.tile([P, KO, S], DT, tag="xt")
            wq_sb = wpool.tile([P, KO, DM], DT, tag="wq")
            wk_sb = wpool.tile([P, KO, DM], DT, tag="wk")
            wv_sb = wpool.tile([P, KO, DM], DT, tag="wv")
            wo_sb = wpool.tile([P, 2, D], DT, tag="wo")
            kt = wpool.tile([P, 2, S], DT, tag="kt")
            v = wpool.tile([P, NSI, H_PER_CORE, HD + 1], DT, tag="v")

            # ---- input DMAs (SP queue, HWDGE) ----
            nc.sync.dma_start(wq_sb[:], Wq.rearrange("(ko p) m -> p ko m", p=P))
            nc.sync.dma_start(wk_sb[:], Wk.rearrange("(ko p) m -> p ko m", p=P))
            nc.sync.dma_start(
                xt[:, :, 0:QT_TILE],
                XTd[:, 0:QT_TILE].rearrange("(ko p) s -> p ko s", p=P),
            )
            nc.sync.dma_start(wv_sb[:], Wv.rearrange("(ko p) m -> p ko m", p=P))
            nc.sync.dma_start(wo_sb[:], Wo.rearrange("(g p) n -> p g n", p=P))
            for qj in range(1, NSJ):
                sl = slice(qj * QT_TILE, (qj + 1) * QT_TILE)
                nc.sync.dma_start(
                    xt[:, :, sl], XTd[:, sl].rearrange("(ko p) s -> p ko s", p=P)
                )

            # ones columns of V (denominator accumulator rows)
            nc.vector.memset(v[:, :, :, HD:HD + 1], 1.0)

            QT_SB = {}   # qj -> qt tile
            CTX_SB = {}  # qj -> ctx sbuf tile

            # ---- psum-group "units" (interleave fillers) ----
            def qk_unit(qj, g, w_sb, dst_ap):
                ps = ps_mm.tile([P, QT_TILE], f32, tag="mm", name="ps_qk")
                sl = slice(qj * QT_TILE, (qj + 1) * QT_TILE)
                for ko in range(KO):
                    nc.tensor.matmul(
                        ps[:], w_sb[:, ko, g * P:(g + 1) * P], xt[:, ko, sl],
                        start=(ko == 0), stop=(ko == KO - 1),
                    )
                nc.vector.tensor_copy(out=dst_ap, in_=ps[:])

            def v_unit(si):
                ps = ps_mm.tile([P, DM], f32, tag="mm", name="ps_v")
                for ko in range(KO):
                    nc.tensor.matmul(
                        ps[:], xt[:, ko, si * P:(si + 1) * P], wv_sb[:, ko, :],
                        start=(ko == 0), stop=(ko == KO - 1),
                    )
                nc.vector.tensor_copy(
                    out=v[:, si, :, 0:HD],
                    in_=ps[:].rearrange("p (h d) -> p h d", d=HD),
                )

            def proj_units(qj):
                qt_sb = qtp.tile([P, 2, QT_TILE], DT, tag="qt", name=f"qt{qj}")
                QT_SB[qj] = qt_sb
                units = []
                for g in range(2):
                    units.append(lambda g=g: qk_unit(qj, g, wq_sb, qt_sb[:, g, :]))
                for g in range(2):
                    units.append(lambda g=g: qk_unit(
                        qj, g, wk_sb,
                        kt[:, g, qj * QT_TILE:(qj + 1) * QT_TILE]))
                for si in range(4 * qj, 4 * qj + 4):
                    units.append(lambda si=si: v_unit(si))
                return units

            def oproj_unit(qj, si, dj):
                ps = ps_mm.tile([P, QT_TILE], f32, tag="mm", name="ps_o")
                sc = (si % 4) * P
                for g in range(2):
                    nc.tensor.matmul(
                        ps[:], CTX_SB[qj][:, g, sc:sc + P],
                        wo_sb[:, g, dj * QT_TILE:(dj + 1) * QT_TILE],
                        start=(g == 0), stop=(g == 1),
                    )
                ot = outp.tile([P, QT_TILE], f16, tag="ot", name="ot")
                nc.vector.tensor_copy(out=ot[:], in_=ps[:])
                nc.sync.dma_start(
                    Out[si * P:(si + 1) * P,
                        dj * QT_TILE:(dj + 1) * QT_TILE],
                    ot[:],
                )

            def oproj_units(qj):
                return [
                    lambda si=si, dj=dj: oproj_unit(qj, si, dj)
                    for si in range(4 * qj, 4 * qj + 4) for dj in range(2)
                ]

            # ---- attention ----
            def attn_block(qj, fillers):
                nk = 4 * qj + 4
                qt_sb = QT_SB[qj]
                ctx_sb = ctxp.tile([P, 2, QT_TILE], DT, tag="ctx",
                                   name=f"ctx{qj}")
                CTX_SB[qj] = ctx_sb
                nfill = len(fillers)
                nsteps = 2 * nk
                state = {"step": 0, "done": 0}

                def maybe_fill():
                    state["step"] += 1
                    want = (state["step"] * nfill) // nsteps
                    while state["done"] < want:
                        fillers[state["done"]]()
                        state["done"] += 1

                for hp in range(2):
                    ctx_ps = [
                        ps_ctx.tile([HD + 1, QT_TILE], f32, tag="ctx",
                                    name=f"ctx_ps{i}")
                        for i in range(2)
                    ]
                    pending = None

                    def scores_exp(ki):
                        j = ki - 4 * qj
                        col0 = max(0, j) * P
                        sps = ps_s.tile([P, 2, QT_TILE], f32, tag="s",
                                        name="s_ps")
                        for i in range(2):
                            hr = 64 * i
                            nc.tensor.matmul(
                                sps[:, i, col0:],
                                kt[hr:hr + 64, hp, ki * P:(ki + 1) * P],
                                qt_sb[hr:hr + 64, hp, col0:],
                                start=True, stop=True,
                            )
                        et = etp.tile([P, 2, QT_TILE], DT, tag="et", name="et")
                        nc.scalar.activation(
                            et[:, :, col0:], sps[:, :, col0:], Exp, scale=0.125
                        )
                        if j >= 0:
                            for i in range(2):
                                nc.vector.tensor_tensor(
                                    et[:, i, col0:col0 + P],
                                    et[:, i, col0:col0 + P], tri[:], mult_op,
                                )
                        return et, col0

                    def pv(ki, et, col0):
                        for i in range(2):
                            h = 2 * hp + i
                            nc.tensor.matmul(
                                ctx_ps[i][:, col0:], v[:, ki, h, :],
                                et[:, i, col0:],
                                start=(ki == 0), stop=(ki == nk - 1),
                            )

                    for ki in range(nk):
                        cur = scores_exp(ki)
                        if pending is not None:
                            pv(*pending)
                        maybe_fill()
                        pending = (ki,) + cur
                    pv(*pending)

                    # normalize both heads of the pair
                    for i in range(2):
                        den = denp.tile([1, QT_TILE], f32, tag="den",
                                        name="den")
                        nc.vector.reciprocal(den[:], ctx_ps[i][HD:HD + 1, :])
                        rbc = rbcp.tile([HD, QT_TILE], f32, tag="rbc",
                                        name="rbc")
                        nc.gpsimd.partition_broadcast(rbc[:], den[:])
                        nc.vector.tensor_tensor(
                            ctx_sb[64 * i:64 * i + 64, hp, :],
                            ctx_ps[i][:HD, :], rbc[:], mult_op,
                        )

                while state["done"] < nfill:
                    fillers[state["done"]]()
                    state["done"] += 1

            # ---- emission: pipelined across q-chunks ----
            for u in proj_units(0):
                u()
            for qj in range(NSJ):
                fillers = []
                if qj > 0:
                    fillers += oproj_units(qj - 1)
                if qj + 1 < NSJ:
                    fillers += proj_units(qj + 1)
                attn_block(qj, fillers)
            for u in oproj_units(NSJ - 1):
                u()

    nc.compile()
    return nc


def _get_nc(mm_mode: str = "bf16"):
    if mm_mode not in _BUILD_CACHE:
        _BUILD_CACHE[mm_mode] = build_bass(mm_mode)
    return _BUILD_CACHE[mm_mode]


def kernel(X, Wq, Wk, Wv, Wo, bo, mm_mode: str = "bf16"):
    from concourse.bass_utils import run_bass_kernel_spmd

    X = np.asarray(X, dtype=np.float32)
    Wq = np.asarray(Wq, dtype=np.float32)
    Wk = np.asarray(Wk, dtype=np.float32)
    Wv = np.asarray(Wv, dtype=np.float32)
    Wo = np.asarray(Wo, dtype=np.float32)
    bo = np.asarray(bo, dtype=np.float32)

    if mm_mode == "bf16":
        import ml_dtypes
        host_dt = np.dtype(ml_dtypes.bfloat16)
    else:
        host_dt = np.dtype(np.float32)

    nc = _get_nc(mm_mode)

    # pre-transpose X per batch: XT [D, S]
    XT = [np.ascontiguousarray(X[b].T).astype(host_dt) for b in range(B)]

    in_maps = []
    for c in range(N_CORES):
        b, g = c // 4, c % 4
        cs = slice(g * DM, (g + 1) * DM)
        in_maps.append({
            "XT": XT[b],
            "Wq": np.ascontiguousarray(Wq[:, cs]).astype(host_dt),
            "Wk": np.ascontiguousarray(Wk[:, cs]).astype(host_dt),
            "Wv": np.ascontiguousarray(Wv[:, cs]).astype(host_dt),
            "Wo": np.ascontiguousarray(Wo[cs, :]).astype(host_dt),
        })

    res = run_bass_kernel_spmd(nc, in_maps, core_ids=list(range(N_CORES)))
    out = np.zeros((B, S, D), dtype=np.float64)
    for c in range(N_CORES):
        out[c // 4] += res.results[c]["Out"].astype(np.float64)
    out += bo.astype(np.float64)
    return out.astype(np.float32)


# revision 7
# speedup vs baseline: 1.3626x; 1.0634x over previous
"""Multi-head causal attention (B=2, S=2048, D=1024, H=16) on 8 TRN2 NeuronCores.

Sharding: batch x head-group tensor parallel. Core c owns batch c//4 and
heads [4*(c%4), 4*(c%4)+4) (a DM=256 model-dim shard). The host
pre-transposes X per batch to XT [D, S] and casts all device inputs to
bf16; each core computes a partial output [S, D] for its batch; the host
sums the 4 partials per batch and adds bo.

Per-core program (all matmul operands bf16, PSUM accumulate f32):
  QT/KT = W^T XT    [256, 2048], m on partitions (2 groups of 128)
  V     = XT^T Wv   [2048, 256] natural (s on partitions), stored per
          head with an appended ones column (softmax denominator trick)
  attention per q-chunk (512) per head pair: scoresT = K Q^T on PE,
  exp on ACT (scale=1/8, both heads of the pair in one activation off a
  2-bank psum tile), causal diagonal-block mask applied multiplicatively
  post-exp on DVE (triangular constant, bf16 4x mode), PV with the ones
  column -> ctxT[64] + denominator row, reciprocal + gpsimd
  partition_broadcast -> normalized ctxT in bf16.
  out[s, :] = ctxT^T Wo (2 dm-chunks accumulated), f16 copy, DMA out.

Emission interleaves projection / out-projection psum groups into the
attention ki-loops (skew-1 scores->PV) so the in-order PE queue never
stalls on the exp->PV dependency.
"""

import numpy as np

B, S, D = 2, 2048, 1024
H_PER_CORE = 4
HD = 64
DM = H_PER_CORE * HD   # 256, per-core model-dim shard
N_CORES = 8
P = 128
QT_TILE = 512          # q free-dim tile in attention
KO = D // P            # 8 contraction chunks for projections
NSI = S // P           # 16 s-chunks of 128
NSJ = S // QT_TILE     # 4 s-chunks of 512

_BUILD_CACHE = {}


def build_bass(mm_mode: str = "bf16"):
    """Build the per-core Bass program. mm_mode in {bf16, fp32r}."""
    import contextlib

    import concourse.tile as tile
    from concourse import bacc, mybir
    from concourse.masks import make_upper_triangular

    f32 = mybir.dt.float32
    f16 = mybir.dt.float16
    DT = mybir.dt.bfloat16 if mm_mode == "bf16" else mybir.dt.float32r
    Exp = mybir.ActivationFunctionType.Exp
    mult_op = mybir.AluOpType.mult

    nc = bacc.Bacc("TRN2", target_bir_lowering=False, debug=False)

    XTd = nc.dram_tensor("XT", [D, S], DT, kind="ExternalInput").ap()
    Wq = nc.dram_tensor("Wq", [D, DM], DT, kind="ExternalInput").ap()
    Wk = nc.dram_tensor("Wk", [D, DM], DT, kind="ExternalInput").ap()
    Wv = nc.dram_tensor("Wv", [D, DM], DT, kind="ExternalInput").ap()
    Wo = nc.dram_tensor("Wo", [DM, D], DT, kind="ExternalInput").ap()
    Out = nc.dram_tensor("Out", [S, D], f16, kind="ExternalOutput").ap()

    lp_ctx = (nc.allow_low_precision(reason="bf16 rounding is intentional")
              if hasattr(nc, "allow_low_precision") else contextlib.nullcontext())
    with lp_ctx, tile.TileContext(nc) as tc:
        with tc.tile_pool(name="consts", bufs=1) as consts, \
             tc.tile_pool(name="wpool", bufs=1) as wpool, \
             tc.tile_pool(name="qt", bufs=2) as qtp, \
             tc.tile_pool(name="et", bufs=6) as etp, \
             tc.tile_pool(name="ctx", bufs=2) as ctxp, \
             tc.tile_pool(name="den", bufs=4) as denp, \
             tc.tile_pool(name="rbc", bufs=3) as rbcp, \
             tc.tile_pool(name="outp", bufs=4) as outp, \
             tc.tile_pool(name="ps_mm", bufs=2, space="PSUM") as ps_mm, \
             tc.tile_pool(name="ps_s", bufs=2, space="PSUM") as ps_s, \
             tc.tile_pool(name="ps_ctx", bufs=2, space="PSUM") as ps_ctx:

            # ---- constants ----
            # tri[k, q] = 1 where k <= q else 0 (keep-mask for the causal
            # diagonal 128x128 block of scoresT)
            tri = consts.tile([P, P], DT, tag="tri")
            make_upper_triangular(nc, tri[:], val=1.0, diag=True)

            # ---- persistent sbuf tensors ----
            xt = wpool.tile([P, KO, S], DT, tag="xt")
            wq_sb = wpool.tile([P, KO, DM], DT, tag="wq")
            wk_sb = wpool.tile([P, KO, DM], DT, tag="wk")
            wv_sb = wpool.tile([P, KO, DM], DT, tag="wv")
            wo_sb = wpool.tile([P, 2, D], DT, tag="wo")
            kt = wpool.tile([P, 2, S], DT, tag="kt")
            v = wpool.tile([P, NSI, H_PER_CORE, HD + 1], DT, tag="v")

            # ---- input DMAs (SP queue, HWDGE); first xt chunk split so the
            # V-projection units can start as early as possible ----
            nc.sync.dma_start(wv_sb[:], Wv.rearrange("(ko p) m -> p ko m", p=P))
            for sl in (slice(0, 256), slice(256, QT_TILE)):
                nc.sync.dma_start(
                    xt[:, :, sl], XTd[:, sl].rearrange("(ko p) s -> p ko s", p=P)
                )
            nc.sync.dma_start(wq_sb[:], Wq.rearrange("(ko p) m -> p ko m", p=P))
            nc.sync.dma_start(wk_sb[:], Wk.rearrange("(ko p) m -> p ko m", p=P))
            nc.sync.dma_start(wo_sb[:], Wo.rearrange("(g p) n -> p g n", p=P))
            for qj in range(1, NSJ):
                sl = slice(qj * QT_TILE, (qj + 1) * QT_TILE)
                nc.sync.dma_start(
                    xt[:, :, sl], XTd[:, sl].rearrange("(ko p) s -> p ko s", p=P)
                )

            # ones columns of V (denominator accumulator rows)
            nc.vector.memset(v[:, :, :, HD:HD + 1], 1.0)

            # warm the Act exp table during the DMA head so the load does
            # not stall the first attention chain
            warm = consts.tile([1, 1], DT, tag="warm")
            nc.scalar.activation(warm[:], tri[0:1, 0:1], Exp, scale=1.0)

            QT_SB = {}   # qj -> qt tile
            CTX_SB = {}  # qj -> ctx sbuf tile

            # ---- psum-group "units" (interleave fillers) ----
            def qk_unit(qj, g, w_sb, dst_ap):
                ps = ps_mm.tile([P, QT_TILE], f32, tag="mm", name="ps_qk")
                sl = slice(qj * QT_TILE, (qj + 1) * QT_TILE)
                for ko in range(KO):
                    nc.tensor.matmul(
                        ps[:], w_sb[:, ko, g * P:(g + 1) * P], xt[:, ko, sl],
                        start=(ko == 0), stop=(ko == KO - 1),
                    )
                nc.vector.tensor_copy(out=dst_ap, in_=ps[:])

            def v_unit(si):
                ps = ps_mm.tile([P, DM], f32, tag="mm", name="ps_v")
                for ko in range(KO):
                    nc.tensor.matmul(
                        ps[:], xt[:, ko, si * P:(si + 1) * P], wv_sb[:, ko, :],
                        start=(ko == 0), stop=(ko == KO - 1),
                    )
                nc.vector.tensor_copy(
                    out=v[:, si, :, 0:HD],
                    in_=ps[:].rearrange("p (h d) -> p h d", d=HD),
                )

            def proj_units(qj):
                qt_sb = qtp.tile([P, 2, QT_TILE], DT, tag="qt", name=f"qt{qj}")
                QT_SB[qj] = qt_sb
                units = []
                for g in range(2):
                    units.append(lambda g=g: qk_unit(qj, g, wq_sb, qt_sb[:, g, :]))
                for g in range(2):
                    units.append(lambda g=g: qk_unit(
                        qj, g, wk_sb,
                        kt[:, g, qj * QT_TILE:(qj + 1) * QT_TILE]))
                for si in range(4 * qj, 4 * qj + 4):
                    units.append(lambda si=si: v_unit(si))
                return units

            def oproj_unit(qj, si, dj):
                ps = ps_mm.tile([P, QT_TILE], f32, tag="mm", name="ps_o")
                sc = (si % 4) * P
                for g in range(2):
                    nc.tensor.matmul(
                        ps[:], CTX_SB[qj][:, g, sc:sc + P],
                        wo_sb[:, g, dj * QT_TILE:(dj + 1) * QT_TILE],
                        start=(g == 0), stop=(g == 1),
                    )
                ot = outp.tile([P, QT_TILE], f16, tag="ot", name="ot")
                nc.vector.tensor_copy(out=ot[:], in_=ps[:])
                nc.sync.dma_start(
                    Out[si * P:(si + 1) * P,
                        dj * QT_TILE:(dj + 1) * QT_TILE],
                    ot[:],
                )

            def oproj_units(qj):
                return [
                    lambda si=si, dj=dj: oproj_unit(qj, si, dj)
                    for si in range(4 * qj, 4 * qj + 4) for dj in range(2)
                ]

            # ---- attention ----
            def attn_block(qj, fillers):
                nk = 4 * qj + 4
                qt_sb = QT_SB[qj]
                ctx_sb = ctxp.tile([P, 2, QT_TILE], DT, tag="ctx",
                                   name=f"ctx{qj}")
                CTX_SB[qj] = ctx_sb
                nfill = len(fillers)
                nsteps = 2 * nk
                state = {"step": 0, "done": 0}

                def maybe_fill(force=0):
                    state["step"] += 1
                    want = (state["step"] * nfill) // nsteps
                    want = max(want, min(state["done"] + force, nfill))
                    while state["done"] < want:
                        fillers[state["done"]]()
                        state["done"] += 1

                for hp in range(2):
                    ctx_ps = [
                        ps_ctx.tile([HD + 1, QT_TILE], f32, tag="ctx",
                                    name=f"ctx_ps{i}")
                        for i in range(2)
                    ]
                    pending = None

                    def scores_exp(ki):
                        j = ki - 4 * qj
                        col0 = max(0, j) * P
                        sps = ps_s.tile([P, 2, QT_TILE], f32, tag="s",
                                        name="s_ps")
                        for i in range(2):
                            hr = 64 * i
                            nc.tensor.matmul(
                                sps[:, i, col0:],
                                kt[hr:hr + 64, hp, ki * P:(ki + 1) * P],
                                qt_sb[hr:hr + 64, hp, col0:],
                                start=True, stop=True,
                            )
                        et = etp.tile([P, 2, QT_TILE], DT, tag="et", name="et")
                        nc.scalar.activation(
                            et[:, :, col0:], sps[:, :, col0:], Exp, scale=0.125
                        )
                        if j >= 0:
                            for i in range(2):
                                nc.vector.tensor_tensor(
                                    et[:, i, col0:col0 + P],
                                    et[:, i, col0:col0 + P], tri[:], mult_op,
                                )
                        return et, col0

                    def pv1(ki, et, col0, i):
                        h = 2 * hp + i
                        nc.tensor.matmul(
                            ctx_ps[i][:, col0:], v[:, ki, h, :],
                            et[:, i, col0:],
                            start=(ki == 0), stop=(ki == nk - 1),
                        )

                    for ki in range(nk):
                        cur = scores_exp(ki)
                        if pending is not None:
                            pv1(*pending, 0)
                            pv1(*pending, 1)
                        maybe_fill(force=2 if ki == 0 else 0)
                        pending = (ki,) + cur

                    # last PV pair interleaved with the normalize chain so
                    # the ctx psum slots free as early as possible
                    den = [denp.tile([1, QT_TILE], f32, tag="den", name="den")
                           for _ in range(2)]
                    rbc = [rbcp.tile([HD, QT_TILE], f32, tag="rbc", name="rbc")
                           for _ in range(2)]
                    pv1(*pending, 0)
                    nc.vector.reciprocal(den[0][:], ctx_ps[0][HD:HD + 1, :])
                    pv1(*pending, 1)
                    nc.vector.reciprocal(den[1][:], ctx_ps[1][HD:HD + 1, :])
                    for i in range(2):
                        nc.gpsimd.partition_broadcast(rbc[i][:], den[i][:])
                    for i in range(2):
                        nc.vector.tensor_tensor(
                            ctx_sb[64 * i:64 * i + 64, hp, :],
                            ctx_ps[i][:HD, :], rbc[i][:], mult_op,
                        )

                while state["done"] < nfill:
                    fillers[state["done"]]()
                    state["done"] += 1

            # ---- emission: pipelined across q-chunks ----
            # prelude: V units first (only need the first half xt chunk + wv)
            u0 = proj_units(0)
            for u in u0[4:] + u0[:4]:
                u()
            for qj in range(NSJ):
                fillers = []
                if qj > 0:
                    fillers += oproj_units(qj - 1)
                if qj + 1 < NSJ:
                    fillers += proj_units(qj + 1)
                attn_block(qj, fillers)
            for u in oproj_units(NSJ - 1):
                u()

    nc.compile()
    return nc


def _get_nc(mm_mode: str = "bf16"):
    if mm_mode not in _BUILD_CACHE:
        _BUILD_CACHE[mm_mode] = build_bass(mm_mode)
    return _BUILD_CACHE[mm_mode]


def kernel(X, Wq, Wk, Wv, Wo, bo, mm_mode: str = "bf16"):
    from concourse.bass_utils import run_bass_kernel_spmd

    X = np.asarray(X, dtype=np.float32)
    Wq = np.asarray(Wq, dtype=np.float32)
    Wk = np.asarray(Wk, dtype=np.float32)
    Wv = np.asarray(Wv, dtype=np.float32)
    Wo = np.asarray(Wo, dtype=np.float32)
    bo = np.asarray(bo, dtype=np.float32)

    if mm_mode == "bf16":
        import ml_dtypes
        host_dt = np.dtype(ml_dtypes.bfloat16)
    else:
        host_dt = np.dtype(np.float32)

    nc = _get_nc(mm_mode)

    # pre-transpose X per batch: XT [D, S]
    XT = [np.ascontiguousarray(X[b].T).astype(host_dt) for b in range(B)]

    in_maps = []
    for c in range(N_CORES):
        b, g = c // 4, c % 4
        cs = slice(g * DM, (g + 1) * DM)
        in_maps.append({
            "XT": XT[b],
            "Wq": np.ascontiguousarray(Wq[:, cs]).astype(host_dt),
            "Wk": np.ascontiguousarray(Wk[:, cs]).astype(host_dt),
            "Wv": np.ascontiguousarray(Wv[:, cs]).astype(host_dt),
            "Wo": np.ascontiguousarray(Wo[cs, :]).astype(host_dt),
        })

    res = run_bass_kernel_spmd(nc, in_maps, core_ids=list(range(N_CORES)))
    out = np.zeros((B, S, D), dtype=np.float64)
    for c in range(N_CORES):
        out[c // 4] += res.results[c]["Out"].astype(np.float64)
    out += bo.astype(np.float64)
    return out.astype(np.float32)
